# revision 17
# baseline (speedup 1.0000x reference)
"""Trainium2 Bass kernel for nn_Attention_8366596292664.

Dense transformer block: qkv proj -> RoPE -> GQA causal attention ->
out proj -> RMSNorm.  B=4, S=2048, H=2048, 16 heads (hd=128), 4 KV heads.

Sharding: 8 cores = (4 batches) x (2 interleaved query-row parities).
Core (b, par) computes the full block for query rows {par, par+2, ...} of
batch b.  Interleaving the query rows by parity makes the causal structure
identical on every core, so one SPMD program serves all 8 cores; the
parity enters only through the data (a 1-column roll of x^T, cos/sin
tables, and the output row scatter).

Structure (all matmuls contract over the partition dim):
  - Phase A: K/V projection, x^T DMA'd in 4 column chunks so matmuls
    start as soon as the first chunk lands (all dynamic DMAs share one
    FIFO queue -> issue order is transfer order).
  - Region B: per q-head, Q projection + RoPE fused with that head's
    attention, so the tensor engine (Q proj + scores/attn@v) outpaces
    the scalar engine (softmax exp LUT) and neither idles.
  - Causal diagonal band: the moving (query) range of the scores /
    attn@v / denominator matmuls is trimmed to the causal boundary
    (partial-width PSUM accumulation; `stop` is sim-only).  Band tiles
    exp into persistent zero-prefix buffers; the 64-wide wedge at the
    boundary is masked by a bf16 multiply on gpsimd.
  - scores^T [k, q]: k-tile stationary, q moving -> softmax along
    partitions; denominator via ones-matmul on pair-summed prob tiles.
  - Phase C: out projection with W_proj streamed in 4 chunks into the
    SBUF freed by x^T (block contraction paced to chunk arrival), then
    RMSNorm with the square-reduce on the vector engine.
"""

import numpy as np
import ml_dtypes

BF16 = ml_dtypes.bfloat16

# ---------------------------------------------------------------- config
P = 128          # partitions
HD = 128         # head dim
HH = HD // 2     # rope half
G = 4            # GQA group size

B = 4
S = 2048
H = 2048
N_CORES = 8

NH = H // HD          # 16 q heads
NKV = NH // G         # 4 kv heads
KVC = NKV * HD        # 512 kv columns
HT = H // P           # 16 h-tiles (contraction tiles)
S_LOC = S // 2        # 1024 local q rows per core
IT = 512              # i-tile (queries per score tile, = 1 psum bank fp32)
NT_I = S_LOC // IT    # 2 i-slots
JB = 8                # band key-tiles per slot
CH = 4                # x column chunks
CHW = S // CH         # 512 columns per chunk
OT = 512              # output-proj column tile
NO = H // OT          # 4

RMS_EPS = 1e-6
SCALE = 1.0 / float(np.sqrt(np.float32(HD)))

_CACHE = {}


# ---------------------------------------------------------------- device IR
def _build_nc():
    from contextlib import ExitStack

    import concourse.bacc as bacc
    import concourse.mybir as mybir
    import concourse.tile as tile

    dt = mybir.dt
    AF = mybir.ActivationFunctionType

    nc = bacc.Bacc("TRN2", target_bir_lowering=False, debug=False)

    xt_d = nc.dram_tensor("xt", [CH, P, HT * CHW], dt.bfloat16, kind="ExternalInput")
    wq_d = nc.dram_tensor("wq", [NH, P, HT, HD], dt.bfloat16, kind="ExternalInput")
    wk_d = nc.dram_tensor("wk", [NKV, P, HT, HD], dt.bfloat16, kind="ExternalInput")
    wv_d = nc.dram_tensor("wv", [P, HT, KVC], dt.bfloat16, kind="ExternalInput")
    wp_d = nc.dram_tensor("wp", [CH, P, (HT // CH) * H], dt.bfloat16, kind="ExternalInput")
    qcos_d = nc.dram_tensor("qcos", [P, S_LOC], dt.bfloat16, kind="ExternalInput")
    qsin_d = nc.dram_tensor("qsin", [P, S_LOC], dt.bfloat16, kind="ExternalInput")
    kcos_d = nc.dram_tensor("kcos", [P, S], dt.bfloat16, kind="ExternalInput")
    ksin_d = nc.dram_tensor("ksin", [P, S], dt.bfloat16, kind="ExternalInput")
    wedge_d = nc.dram_tensor("wedge", [P, 128], dt.bfloat16, kind="ExternalInput")
    nw_d = nc.dram_tensor("nw", [P, H], dt.float32, kind="ExternalInput")
    out_d = nc.dram_tensor("out", [S_LOC, H], dt.float32, kind="ExternalOutput")

    import os
    dbg = os.environ.get("KDBG") == "1"
    gps = nc.gpsimd if os.environ.get("KGPS") == "1" else nc.vector
    if dbg:
        dqT_d = nc.dram_tensor("dqT", [P, NH * S_LOC], dt.bfloat16, kind="ExternalOutput")
        dkT_d = nc.dram_tensor("dkT", [P, NKV * S], dt.bfloat16, kind="ExternalOutput")
        dvv_d = nc.dram_tensor("dvv", [P, NKV * (S // P) * HD], dt.bfloat16, kind="ExternalOutput")
        dyT_d = nc.dram_tensor("dyT", [P, NH * S_LOC], dt.bfloat16, kind="ExternalOutput")

    with tile.TileContext(nc) as tc, ExitStack() as body:
        const = body.enter_context(tc.tile_pool(name="const", bufs=1))
        onesm = const.tile([P, P], dt.bfloat16)
        nc.vector.memset(onesm[:], 1.0)
        epsb = const.tile([P, 1], dt.float32)
        nc.vector.memset(epsb[:], RMS_EPS)
        warm = const.tile([P, 1], dt.float32)
        nc.vector.memset(warm[:], 0.0)
        # pre-load the exp LUT set while the startup DMAs stream
        nc.scalar.activation(warm[:], warm[:], AF.Exp)

        s_act = body.enter_context(ExitStack())
        act = s_act.enter_context(tc.tile_pool(name="act", bufs=1))
        qT = act.tile([P, NH * S_LOC], dt.bfloat16)
        kT = act.tile([P, NKV * S], dt.bfloat16)
        # v in kv-head-major layout: col = kvh*(S//P)*HD + sv*HD + d
        vv = act.tile([P, NKV * (S // P) * HD], dt.bfloat16)

        s_x = body.enter_context(ExitStack())
        xp = s_x.enter_context(tc.tile_pool(name="xp", bufs=1))
        xt = xp.tile([P, CH * HT * CHW], dt.bfloat16)
        trigq = s_x.enter_context(tc.tile_pool(name="trigq", bufs=1))
        qcos = trigq.tile([P, S_LOC], dt.bfloat16)
        qsin = trigq.tile([P, S_LOC], dt.bfloat16)

        def rope_evict(rpool, ps, dst_lo, dst_hi, cs, sn):
            # dst_lo = ps_lo*cos - ps_hi*sin ; dst_hi = ps_hi*cos + ps_lo*sin
            w = ps.shape[1]
            stg = rpool.tile([P, w], dt.bfloat16, name="rstg")
            nc.vector.tensor_copy(stg[:], ps[:])
            t1 = rpool.tile([HH, w], dt.bfloat16, name="rt1")
            t2 = rpool.tile([HH, w], dt.bfloat16, name="rt2")
            nc.vector.tensor_mul(t1[:], stg[0:HH, :], cs[0:HH, :])
            nc.vector.tensor_mul(t2[:], stg[HH:P, :], sn[HH:P, :])
            nc.vector.tensor_sub(dst_lo, t1[:], t2[:])
            nc.vector.tensor_mul(t1[:], stg[HH:P, :], cs[HH:P, :])
            nc.vector.tensor_mul(t2[:], stg[0:HH, :], sn[0:HH, :])
            nc.vector.tensor_add(dst_hi, t1[:], t2[:])

        # ---------------- phase A: k/v projection + k rope -------------
        with ExitStack() as pha:
            trigk = pha.enter_context(tc.tile_pool(name="trigk", bufs=1))
            kcos = trigk.tile([P, S], dt.bfloat16)
            ksin = trigk.tile([P, S], dt.bfloat16)
            wkp = pha.enter_context(tc.tile_pool(name="wkp", bufs=1))
            wk = wkp.tile([P, NKV * HT * HD], dt.bfloat16)
            wvp = pha.enter_context(tc.tile_pool(name="wvp", bufs=1))
            wv = wvp.tile([P, HT * KVC], dt.bfloat16)

            # DMA issue order == transfer order (single HW queue)
            xt3 = xt[:].rearrange("p (t s) -> p t s", t=HT)

            def dma_x_chunk(c, lo=0, hi=CHW):
                nc.sync.dma_start(
                    xt3[:, :, c * CHW + lo : c * CHW + hi],
                    xt_d.ap()[c].rearrange("p (t s) -> p t s", t=HT)[:, :, lo:hi],
                )

            wk4 = wk[:].rearrange("p (f t m) -> p f t m", f=NKV, t=HT)
            nc.sync.dma_start(wk4[:, 0], wk_d.ap()[0])
            dma_x_chunk(0, 0, CHW // 2)
            nc.sync.dma_start(wk4[:, 1], wk_d.ap()[1])
            dma_x_chunk(0, CHW // 2, CHW)
            for fk in range(2, NKV):
                nc.sync.dma_start(wk4[:, fk], wk_d.ap()[fk])
            nc.sync.dma_start(kcos[:], kcos_d.ap())
            nc.sync.dma_start(ksin[:], ksin_d.ap())
            nc.sync.dma_start(wv[:], wv_d.ap())
            for c in range(1, CH):
                dma_x_chunk(c)
            nc.sync.dma_start(qcos[:], qcos_d.ap())
            nc.sync.dma_start(qsin[:], qsin_d.ap())

            rpk = pha.enter_context(tc.tile_pool(name="rpk", bufs=2))
            psk = pha.enter_context(tc.tile_pool(name="psk", bufs=2, space="PSUM"))
            psv = pha.enter_context(tc.tile_pool(name="psv", bufs=2, space="PSUM"))

            vv4 = vv[:].rearrange("p (k j d) -> p k j d", k=NKV, j=S // P)

            for c in range(CH):
                halves = [(0, CHW // 2), (CHW // 2, CHW)] if c == 0 else [(0, CHW)]
                for lo, hi in halves:
                    for fk in range(NKV):
                        w = hi - lo
                        ps = psk.tile([P, w], dt.float32, name="kps")
                        for h in range(HT):
                            nc.tensor.matmul(
                                ps[:],
                                wk[:, fk * H + h * HD : fk * H + (h + 1) * HD],
                                xt[:, h * S + c * CHW + lo : h * S + c * CHW + hi],
                                start=(h == 0),
                                stop=(h == HT - 1),
                            )
                        c0 = fk * S + c * CHW + lo
                        rope_evict(
                            rpk, ps,
                            kT[0:HH, c0 : c0 + w], kT[HH:P, c0 : c0 + w],
                            kcos[:, c * CHW + lo : c * CHW + hi],
                            ksin[:, c * CHW + lo : c * CHW + hi],
                        )
                for sv in range(4 * c, 4 * c + 4):
                    ps = psv.tile([P, KVC], dt.float32, name="vps")
                    for h in range(HT):
                        nc.tensor.matmul(
                            ps[:],
                            xt[:, h * S + sv * P : h * S + (sv + 1) * P],
                            wv[:, h * KVC : (h + 1) * KVC],
                            start=(h == 0),
                            stop=(h == HT - 1),
                        )
                    nc.scalar.activation(
                        vv4[:, :, sv, :],
                        ps[:].rearrange("p (k d) -> p k d", k=NKV),
                        AF.Copy,
                    )

        # ---------------- region B: per-head q proj + attention --------
        late = body.enter_context(tc.tile_pool(name="late", bufs=1, side="right"))
        wedge = late.tile([P, 128], dt.bfloat16)
        nc.sync.dma_start(wedge[:], wedge_d.ap())
        nw = late.tile([P, H], dt.float32)
        nc.sync.dma_start(nw[:], nw_d.ap())
        yT = late.tile([P, NH * S_LOC], dt.bfloat16)
        # persistent band prob buffers; zero prefixes survive reuse
        # padded by 512 cols so the fused pair-wedge strided view stays in range
        band = late.tile([P, JB * IT + IT], dt.bfloat16)
        nc.vector.memset(band[:], 0.0)

        with ExitStack() as phb:
            wqp = phb.enter_context(tc.tile_pool(name="wqp", bufs=2))
            rpq = phb.enter_context(tc.tile_pool(name="rpq", bufs=2))
            prp = phb.enter_context(tc.tile_pool(name="prp", bufs=3))
            dsp = phb.enter_context(tc.tile_pool(name="dsp", bufs=4))
            recp = phb.enter_context(tc.tile_pool(name="recp", bufs=1))
            psq = phb.enter_context(tc.tile_pool(name="psq", bufs=2, space="PSUM"))
            pss_p = phb.enter_context(tc.tile_pool(name="pssp", bufs=2, space="PSUM"))
            psy = phb.enter_context(tc.tile_pool(name="psy", bufs=1, space="PSUM"))
            psd = phb.enter_context(tc.tile_pool(name="psd", bufs=1, space="PSUM"))

            def emit_qproj(fq):
                wq = wqp.tile([P, HT * HD], dt.bfloat16, name="wqt")
                nc.sync.dma_start(
                    wq[:].rearrange("p (t m) -> p t m", t=HT),
                    wq_d.ap()[fq],
                )
                pss = [psq.tile([P, IT], dt.float32, name="qps") for _ in range(NT_I)]
                for h in range(HT):
                    for t in range(NT_I):
                        st = h * S + 2 * t * IT
                        nc.tensor.matmul(
                            pss[t][:],
                            wq[:, h * HD : (h + 1) * HD],
                            xt[:, st : st + 2 * IT : 2],
                            start=(h == 0),
                            stop=(h == HT - 1),
                        )
                for t in range(NT_I):
                    c0 = fq * S_LOC + t * IT
                    rope_evict(
                        rpq, pss[t],
                        qT[0:HH, c0 : c0 + IT], qT[HH:P, c0 : c0 + IT],
                        qcos[:, t * IT : (t + 1) * IT],
                        qsin[:, t * IT : (t + 1) * IT],
                    )

            def emit_attn(hq):
                kvh = hq // G
                kbase = kvh * S
                vbase = kvh * (S // P) * HD
                for t in range(NT_I):
                    qsl = qT[:, hq * S_LOC + t * IT : hq * S_LOC + (t + 1) * IT]
                    yps = psy.tile([P, IT], dt.float32, name="yps")
                    dps = psd.tile([P, IT], dt.float32, name="dps")

                    # units: nonband pairs (slot 1 only), then band pairs.
                    units = []
                    for m in range(t * 4):
                        units.append(
                            ("nb", [(2 * m, 0, None), (2 * m + 1, 0, None)])
                        )
                    for m in range(4):
                        units.append(
                            ("bd", [
                                (t * JB + 2 * m, 128 * m, 2 * m),
                                (t * JB + 2 * m + 1, 128 * m + 64, 2 * m + 1),
                            ])
                        )
                    nu = len(units)
                    nj = (t + 1) * JB

                    def emit_scores(u):
                        kind, tiles = u
                        sps = pss_p.tile([P, 2 * IT], dt.float32, name="sps")
                        for ui, (j, c0, jj) in enumerate(tiles):
                            nc.tensor.matmul(
                                sps[:, ui * IT + c0 : (ui + 1) * IT],
                                kT[:, kbase + j * P : kbase + (j + 1) * P],
                                qsl[:, c0:IT],
                                start=True,
                                stop=True,
                                skip_group_check=True,
                            )
                        return sps

                    sps_q = [emit_scores(units[0])]
                    for i in range(nu):
                        kind, tiles = units[i]
                        sps = sps_q.pop(0)
                        if i + 1 < nu:
                            sps_q.append(emit_scores(units[i + 1]))
                        first, last = (i == 0), (i == nu - 1)
                        if kind == "nb":
                            pr = prp.tile([P, 2 * IT], dt.bfloat16, name="pr")
                            nc.scalar.activation(pr[:], sps[:], AF.Exp, scale=SCALE)
                            ds = dsp.tile([P, IT], dt.bfloat16, name="ds")
                            gps.tensor_add(
                                ds[:], pr[:, 0:IT], pr[:, IT : 2 * IT]
                            )
                            for ui, (j, c0, jj) in enumerate(tiles):
                                nc.tensor.matmul(
                                    yps[:],
                                    vv[:, vbase + j * HD : vbase + (j + 1) * HD],
                                    pr[:, ui * IT : (ui + 1) * IT],
                                    start=(j == 0),
                                    stop=(j == nj - 1),
                                    skip_group_check=True,
                                )
                            nc.tensor.matmul(
                                dps[:], onesm[:], ds[:],
                                start=first, stop=last,
                                skip_group_check=True,
                            )
                        else:
                            c0a = tiles[0][1]
                            for ui, (j, c0, jj) in enumerate(tiles):
                                # exact LUT exp into the zero-prefix band buf
                                nc.scalar.activation(
                                    band[:, jj * IT + c0 : (jj + 1) * IT],
                                    sps[:, ui * IT + c0 : (ui + 1) * IT],
                                    AF.Exp, scale=SCALE,
                                )
                            # both wedges of the pair in one strided op
                            wa = tiles[0][2] * IT + c0a
                            wview = band[:, wa : wa + 2 * (IT + 64)].rearrange(
                                "p (j c) -> p j c", j=2
                            )[:, :, 0:64]
                            gps.tensor_mul(
                                wview, wview,
                                wedge[:].rearrange("p (j c) -> p j c", j=2),
                            )
                            ds = dsp.tile([P, IT], dt.bfloat16, name="ds")
                            jja, jjb = tiles[0][2], tiles[1][2]
                            (nc.vector if jja % 4 == 0 else gps).tensor_add(
                                ds[:, c0a:IT],
                                band[:, jja * IT + c0a : (jja + 1) * IT],
                                band[:, jjb * IT + c0a : (jjb + 1) * IT],
                            )
                            for ui, (j, c0, jj) in enumerate(tiles):
                                nc.tensor.matmul(
                                    yps[:, c0:IT],
                                    vv[:, vbase + j * HD : vbase + (j + 1) * HD],
                                    band[:, jj * IT + c0 : (jj + 1) * IT],
                                    start=(j == 0),
                                    stop=(j == nj - 1),
                                    skip_group_check=True,
                                )
                            nc.tensor.matmul(
                                dps[:, c0a:IT], onesm[:], ds[:, c0a:IT],
                                start=first, stop=last,
                                skip_group_check=True,
                            )

                    rec = recp.tile([P, IT], dt.float32, name="rec")
                    nc.vector.reciprocal_approx_fast(rec[:], dps[:])
                    nc.vector.tensor_mul(
                        yT[:, hq * S_LOC + t * IT : hq * S_LOC + (t + 1) * IT],
                        yps[:],
                        rec[:],
                    )

            emit_qproj(0)
            for hq in range(NH):
                if hq + 1 < NH and hq < 13:
                    emit_qproj(hq + 1)
                elif hq == 13:
                    emit_qproj(14)
                    emit_qproj(15)
                emit_attn(hq)

        if dbg:
            nc.sync.dma_start(dqT_d.ap(), qT[:])
            nc.sync.dma_start(dkT_d.ap(), kT[:])
            nc.sync.dma_start(dvv_d.ap(), vv[:])
            nc.sync.dma_start(dyT_d.ap(), yT[:])

        s_x.close()   # free x^T / q trig -> space for streamed W_proj
        s_act.close() # free qT / kT / vv

        # ---------------- phase C: out projection + rmsnorm ------------
        with ExitStack() as phc:
            wpp = phc.enter_context(tc.tile_pool(name="wpp", bufs=1))
            wp = wpp.tile([P, HT * H], dt.bfloat16)
            # stream in 4 chunks (h-tiles 4c..4c+3 each); phase C's
            # h-ascending contraction paces with chunk arrival
            hc = HT // CH
            for c in range(CH):
                nc.sync.dma_start(
                    wp[:, c * hc * H : (c + 1) * hc * H], wp_d.ap()[c]
                )
            outp = phc.enter_context(tc.tile_pool(name="outp", bufs=2))
            sqp = phc.enter_context(tc.tile_pool(name="sqp", bufs=2))
            smp = phc.enter_context(tc.tile_pool(name="smp", bufs=8))
            po = phc.enter_context(tc.tile_pool(name="po", bufs=8, space="PSUM"))

            for sl in range(S_LOC // P):
                pso = [po.tile([P, OT], dt.float32, name="pso") for _ in range(NO)]
                for h in range(HT):
                    lhs = yT[:, h * S_LOC + sl * P : h * S_LOC + (sl + 1) * P]
                    for o in range(NO):
                        nc.tensor.matmul(
                            pso[o][:],
                            lhs,
                            wp[:, h * H + o * OT : h * H + (o + 1) * OT],
                            start=(h == 0),
                            stop=(h == HT - 1),
                        )
                ot = outp.tile([P, H], dt.float32, name="ot")
                sq = sqp.tile([P, OT], dt.float32, name="sq")
                parts = [smp.tile([P, 1], dt.float32, name="ssq") for _ in range(NO)]
                for o in range(NO):
                    osl = slice(o * OT, (o + 1) * OT)
                    nc.scalar.activation(ot[:, osl], pso[o][:], AF.Copy)
                    nc.scalar.activation(
                        sq[:], ot[:, osl], AF.Square, accum_out=parts[o][:]
                    )
                s01 = smp.tile([P, 1], dt.float32, name="s01")
                s23 = smp.tile([P, 1], dt.float32, name="s23")
                ssq = smp.tile([P, 1], dt.float32, name="ssqt")
                nc.vector.tensor_add(s01[:], parts[0][:], parts[1][:])
                nc.vector.tensor_add(s23[:], parts[2][:], parts[3][:])
                nc.vector.tensor_add(ssq[:], s01[:], s23[:])
                rms = smp.tile([P, 1], dt.float32, name="rms")
                nc.scalar.activation(
                    rms[:], ssq[:], AF.Sqrt, bias=epsb[:], scale=1.0 / H
                )
                rr = smp.tile([P, 1], dt.float32, name="rr")
                nc.vector.reciprocal(rr[:], rms[:])
                fin = outp.tile([P, H], dt.float32, name="fin")
                for o in range(NO):
                    osl = slice(o * OT, (o + 1) * OT)
                    # fin = (ot * rr) * nw in one vector op
                    nc.vector.scalar_tensor_tensor(
                        fin[:, osl], ot[:, osl], rr[:], nw[:, osl],
                        mybir.AluOpType.mult, mybir.AluOpType.mult,
                    )
                    nc.sync.dma_start(
                        out_d.ap()[sl * P : (sl + 1) * P, osl], fin[:, osl]
                    )

    nc.compile()
    return nc


# ---------------------------------------------------------------- host side
def _host_shared(w_attn, w_proj, norm_w):
    """Core-independent packed tensors."""
    f32 = np.float32

    def perm_halves(w):  # [H, n, HD] even/odd pairs -> halves
        return np.concatenate([w[..., 0::2], w[..., 1::2]], axis=-1)

    wq = perm_halves(w_attn[:, :H].reshape(H, NH, HD))
    wq = np.ascontiguousarray(
        wq.reshape(HT, P, NH, HD).transpose(2, 1, 0, 3)
    ).astype(BF16)
    wk = perm_halves(w_attn[:, H : H + KVC].reshape(H, NKV, HD))
    wk = np.ascontiguousarray(
        wk.reshape(HT, P, NKV, HD).transpose(2, 1, 0, 3)
    ).astype(BF16)
    wv = np.ascontiguousarray(
        w_attn[:, H + KVC :].reshape(HT, P, KVC).transpose(1, 0, 2)
    ).astype(BF16)
    # wp chunked by groups of 4 h-tiles: [CH, P, 4*H]
    wp = np.ascontiguousarray(
        w_proj.reshape(CH, HT // CH, P, H).transpose(0, 2, 1, 3).reshape(CH, P, (HT // CH) * H)
    ).astype(BF16)

    p, f = np.meshgrid(np.arange(P), np.arange(64), indexing="ij")
    # wedge masks within a band tile; independent of jj (shipped twice for
    # the fused pair-wedge op)
    wedge0 = np.tile((2 * f >= p).astype(BF16), (1, 2))          # parity 0
    wedge1 = np.tile((2 * f + 1 >= (p ^ 1)).astype(BF16), (1, 2))  # parity 1

    nw = np.ascontiguousarray(
        np.broadcast_to(norm_w.astype(f32), (P, H))
    )
    return wq, wk, wv, wp, (wedge0, wedge1), nw


def _cos_sin(pos):
    f32 = np.float32
    inv = 1.0 / (
        10000.0 ** (np.arange(0, HD, 2, dtype=f32) / f32(HD))
    )
    ang = inv[:, None].astype(f32) * pos[None, :].astype(f32)  # [HH, N]
    c, s = np.cos(ang).astype(BF16), np.sin(ang).astype(BF16)
    # duplicated across both partition halves (walrus wants equal base
    # partitions for SBUF tensor-tensor inputs)
    return (
        np.ascontiguousarray(np.concatenate([c, c], axis=0)),
        np.ascontiguousarray(np.concatenate([s, s], axis=0)),
    )


def make_in_maps(x, w_attn, w_proj, norm_w):
    x = np.asarray(x, dtype=np.float32)
    w_attn = np.asarray(w_attn, dtype=np.float32)
    w_proj = np.asarray(w_proj, dtype=np.float32)
    norm_w = np.asarray(norm_w, dtype=np.float32)

    wq, wk, wv, wp, (wedge0, wedge1), nw = _host_shared(w_attn, w_proj, norm_w)

    kc0, ks0 = _cos_sin(np.arange(S, dtype=np.float32))          # parity 0
    # parity 1: column j holds global row j^1 (pair-swapped x columns)
    kc1, ks1 = _cos_sin((np.arange(S) ^ 1).astype(np.float32))
    qc0, qs0 = _cos_sin(2.0 * np.arange(S_LOC, dtype=np.float32))
    qc1, qs1 = _cos_sin(2.0 * np.arange(S_LOC, dtype=np.float32) + 1.0)

    in_maps = []
    for c in range(N_CORES):
        b, par = c // 2, c % 2
        xt = x[b].T.astype(BF16)
        if par:
            xt = xt[:, np.arange(S) ^ 1]  # swap adjacent column pairs
        # chunk-major packing: [CH, P, HT*CHW]
        xt = np.ascontiguousarray(
            xt.reshape(HT, P, CH, CHW).transpose(2, 1, 0, 3).reshape(CH, P, HT * CHW)
        )
        in_maps.append(
            {
                "xt": xt,
                "wq": wq,
                "wk": wk,
                "wv": wv,
                "wp": wp,
                "qcos": qc1 if par else qc0,
                "qsin": qs1 if par else qs0,
                "kcos": kc1 if par else kc0,
                "ksin": ks1 if par else ks0,
                "wedge": wedge1 if par else wedge0,
                "nw": nw,
            }
        )
    return in_maps


def assemble_out(results):
    out = np.empty((B, S, H), dtype=np.float32)
    for c in range(N_CORES):
        b, par = c // 2, c % 2
        out[b, par::2, :] = results[c]["out"]
    return out


def kernel(x, w_attn, w_proj, norm_w):
    from concourse import bass_utils

    if "nc" not in _CACHE:
        _CACHE["nc"] = _build_nc()
    nc = _CACHE["nc"]

    in_maps = make_in_maps(x, w_attn, w_proj, norm_w)
    res = bass_utils.run_bass_kernel_spmd(
        nc, in_maps, core_ids=list(range(N_CORES))
    )
    return assemble_out(res.results)
